# revision 1
# baseline (speedup 1.0000x reference)
"""CTC loss kernel for Trainium2 (8 NeuronCores, data-parallel over batch).

Algorithm (column-scan CTC):
  reference loss = -logaddexp(a[il-1, 2ll], a[il-1, 2ll-1]) where a = CTC
  forward DP in log space over logp = log_softmax(log(y_pred+eps)).

  Identities used:
   * log_softmax(log(q)) = log(q) - log(sum_c q), q = y_pred + eps
   * Run the DP in LINEAR space on blank-RATIOS r[t,s] = q[t,lab_s]/q[t,blank].
     Then alpha_hat[t,s] = alpha[t,s] / prod_{tau<=t} q[tau,blank]; the all-blank
     path is exactly 1, and each state accumulates at most L=100 label-ratio
     factors => fp32 range is safe with NO renormalization.
   * s-major sweep: column s of the DP over all t is a first-order affine
     recurrence  state = (u[t] + state) * r[t]  -> native DVE
     tensor_tensor_scan. Even (blank) columns have r == 1 => plain cumsum.
   * One appended all-blank pad frame makes the final blank state at t=T
     equal alpha[il-1, 2ll] + alpha[il-1, 2ll-1] (both readout terms merged),
     for ANY per-sample il, after masking ratios for t >= il to zero.
  loss = -( log(E_ll[T]) + sum_t log q_blank[t] - sum_t log denom[t] ),
  sums over t < il (host bakes neutral frames: qb=1, denom=1, r=0 beyond il).

Device per core (64 samples on partitions 0..63):
  pairs i=0..100:  E_i = cumsum-scan(O_{i-1});  u = stt(E_i, m_i, O_{i-1});
                   O_i = affine-scan(u, r_i)        (E_0 = ones, no O_100)
  PE: denom[t] = ones @ Y^T chunks; ACT: Ln; PE: sum_t; ACT: Ln(qb) accum.
Host: layout prep (transpose/cast/gather+ratio), final log + combine.
"""
import sys
import types
import json
import numpy as np
import ml_dtypes

EPS = 1e-7
B, T, C = 512, 512, 96
L = 100
NCORE = 8
BS = B // NCORE          # 64 samples per core
TP = T + 1               # +1 all-blank pad frame
NP = L + 1               # column pairs 0..100 (E_100 readout, O_100 unused)
BLANK = C - 1

bf16 = ml_dtypes.bfloat16

SKEW = True          # 2-way time-chunk pipeline skew across partitions
CH = 257             # chunk width (chunk1: t 0..256; chunk2: t 257..512 + pad)
LAG = 4              # stream lag between chunk1 and chunk2 of a pair
NSTREAM = NP + LAG   # 105 stream steps

_BUILT = {}


def _install_axon_profile_hook():
    """Make run_bass_kernel_spmd(trace=True) usable under axon (optional)."""
    try:
        if "antenv.axon_hooks" in sys.modules:
            return
        import antenv  # noqa: F401
        from trn_agent_boot.trn_boot import _ntff_profile_via_ctypes
        hook = _ntff_profile_via_ctypes('/opt/axon/libaxon_pjrt.so')
        mod = types.ModuleType("antenv.axon_hooks")
        mod.get_axon_ntff_profile_hook = lambda: hook
        mod.set_axon_ntff_profile_hook = lambda h: None
        sys.modules["antenv.axon_hooks"] = mod
    except Exception:
        pass


def _install_birfix():
    """Cap sync waits per instruction for the nix walrus_driver: insert NoOps
    carrying excess waits immediately before the instruction (same engine)."""
    import concourse.bass_utils as bu
    import concourse.bass2jax as b2j
    if getattr(bu, "_ctc_birfix", False):
        return
    orig = bu.compile_bir_kernel

    def _legalize(bir_json: bytes, limit: int = 1) -> bytes:
        bir = json.loads(bir_json)
        n = 0
        changed = False
        for fn in bir.get("functions", []):
            for blk in fn.get("blocks", []):
                out = []
                for ins in blk.get("instructions", []):
                    si = ins.get("sync_info")
                    waits = (si or {}).get("on_wait") or []
                    if len(waits) > limit:
                        extra, keep = waits[:-limit], waits[-limit:]
                        for k in range(0, len(extra), limit):
                            n += 1
                            out.append({
                                "engine": ins["engine"], "ins": [],
                                "name": f"wsplit-nop-{n}", "opcode": "NoOp",
                                "outs": [],
                                "sync_info": {"on_update": [],
                                              "on_wait": extra[k:k + limit]},
                            })
                        si["on_wait"] = keep
                        changed = True
                    out.append(ins)
                blk["instructions"] = out
        return json.dumps(bir).encode() if changed else bir_json

    def patched(bir_json, tmpdir, neff_name="file.neff"):
        return orig(_legalize(bir_json), tmpdir, neff_name)

    bu.compile_bir_kernel = patched
    b2j.compile_bir_kernel = patched
    bu._ctc_birfix = True


def _build_program_skew():
    """Skewed build: all DVE ops [128, CH]. Rows 0..63 process chunk1 of
    stream-pair k; rows 64..127 process chunk2 of pair k-LAG. Chunk-boundary
    state crosses partitions via tiny PE shift-matmuls with LAG slack."""
    import concourse.bass as bass
    import concourse.mybir as mybir
    import concourse.tile as tile

    f32 = mybir.dt.float32
    b16 = mybir.dt.bfloat16
    ALU = mybir.AluOpType
    ACTF = mybir.ActivationFunctionType

    nc = bass.Bass()
    rat_d = nc.dram_tensor("rat2", [128, NSTREAM, CH], b16, kind="ExternalInput")
    dr2_d = nc.dram_tensor("dr2", [128, CH], b16, kind="ExternalInput")
    drf_d = nc.dram_tensor("drf", [BS, TP], b16, kind="ExternalInput")
    qb_d = nc.dram_tensor("qb", [BS, TP], f32, kind="ExternalInput")
    m_d = nc.dram_tensor("msk2", [128, NSTREAM], f32, kind="ExternalInput")
    yt_d = nc.dram_tensor("yt", [C, BS * TP], b16, kind="ExternalInput")
    sh_d = nc.dram_tensor("sh", [BS, 128], f32, kind="ExternalInput")
    out_d = nc.dram_tensor("out", [128, NP + 2], f32, kind="ExternalOutput")

    TCH = 4
    NB = 8

    with tile.TileContext(nc) as tc:
        with (
            tc.tile_pool(name="pool", bufs=1) as pool,
            tc.tile_pool(name="psum", bufs=1, space="PSUM") as psum,
        ):
            rat = pool.tile([128, NSTREAM * CH], b16)
            dr2 = pool.tile([128, CH], b16)
            drf = pool.tile([BS, TP], b16)
            qb = pool.tile([BS, TP], f32)
            msk = pool.tile([128, NSTREAM], f32)
            yt = pool.tile([C, BS * TP], b16)
            sh = pool.tile([BS, 128], f32)
            ones96 = pool.tile([C, 1], b16)
            zrow = pool.tile([BS, TP], b16)
            col0f = pool.tile([BS, 1 + TP], f32)
            col0sk = pool.tile([128, 2 + CH], f32)
            u2 = pool.tile([128, CH], f32)
            obufs = [pool.tile([128, 2 + CH], f32, name=f"ob{i}", tag=f"ob{i}")
                     for i in range(NB)]
            ebufs = [pool.tile([128, 2 + CH], f32, name=f"eb{i}", tag=f"eb{i}")
                     for i in range(NB)]
            lnqb = pool.tile([BS, TP], f32)
            ld = [pool.tile([128, BS], f32, name=f"ld{i}", tag=f"ld{i}")
                  for i in range(TCH)]
            res = pool.tile([128, NP + 2], f32)
            ones128 = pool.tile([128, 1], f32)
            eps96 = pool.tile([128, 1], f32)
            pden = [psum.tile([128, BS], f32, name=f"pd{i}", tag=f"pd{i}")
                    for i in range(2)]
            psums = psum.tile([BS, 1], f32)
            pcol = psum.tile([128, CH], f32)
            phop = [psum.tile([128, 1], f32, name=f"ph{i}", tag=f"ph{i}")
                    for i in range(2)]

            # --- loads ---
            nc.gpsimd.dma_start(msk[:], m_d[:])
            nc.gpsimd.dma_start(qb[:], qb_d[:])
            nc.gpsimd.dma_start(dr2[:], dr2_d[:])
            nc.gpsimd.dma_start(drf[:], drf_d[:])
            nc.gpsimd.dma_start(sh[:], sh_d[:])
            nc.gpsimd.dma_start(yt[:], yt_d[:])
            NRC = 15
            step = (NSTREAM + NRC - 1) // NRC
            for k in range(NRC):
                lo = k * step
                hi = min(NSTREAM, lo + step)
                if lo >= hi:
                    continue
                nc.gpsimd.dma_start(
                    rat[:, lo * CH:hi * CH],
                    rat_d[:, lo:hi, :].rearrange("b l t -> b (l t)"))

            # --- init ---
            nc.vector.memset(zrow[:], 0.0)
            nc.vector.memset(res[:], 1.0)
            nc.vector.memset(ones96[:], 1.0)
            nc.vector.memset(ones128[:], 1.0)
            nc.vector.memset(eps96[:], float(C) * EPS)
            nc.vector.memset(col0f[:, 0:1], 1.0)
            nc.vector.memset(col0sk[:], 0.0)
            for ob in obufs:
                nc.vector.memset(ob[:], 0.0)
            for eb in ebufs:
                nc.vector.memset(eb[:], 0.0)

            # --- sum_t log qb ---
            nc.scalar.activation(lnqb[:], qb[:], ACTF.Ln,
                                 accum_out=res[0:BS, NP:NP + 1])

            # --- denominators ---
            for k in range(TCH):
                for b in range(BS):
                    base = b * TP + 128 * k
                    nc.tensor.matmul(
                        pden[k % 2][:, b:b + 1], yt[:, base:base + 128],
                        ones96[:], start=True, stop=True)
                nc.scalar.activation(ld[k][:], pden[k % 2][:], ACTF.Ln,
                                     bias=eps96[:])
            for k in range(TCH):
                nc.tensor.matmul(psums[:], ld[k][:], ones128[:],
                                 start=(k == 0), stop=(k == TCH - 1))
            nc.scalar.copy(res[0:BS, NP + 1:NP + 2], psums[:])

            # --- col 0 (all-blank) full-length, then skewed assembly ---
            nc.vector.tensor_tensor_scan(
                col0f[:, 1:1 + TP], zrow[:, 0:TP], drf[:, 0:TP], 1.0,
                op0=ALU.add, op1=ALU.mult)
            nc.scalar.copy(col0sk[0:BS, 1:1 + 258], col0f[:, 0:258])
            nc.tensor.matmul(pcol[:, 0:257], sh[:], col0f[:, 257:514],
                             start=True, stop=True)
            nc.scalar.copy(col0sk[BS:128, 1:258], pcol[BS:128, 0:257])

            # --- skewed DP stream ---
            for k in range(NSTREAM):
                ob = obufs[k % NB]
                obp = obufs[(k - 1) % NB]
                eb = ebufs[k % NB]
                if k >= 1:
                    nc.vector.tensor_tensor_scan(
                        eb[:, 2:2 + CH], obp[:, 1:1 + CH], dr2[:, 0:CH],
                        eb[:, 0:1], op0=ALU.add, op1=ALU.mult)
                    if k >= LAG + 1:
                        nc.scalar.copy(res[BS:128, (k - LAG):(k - LAG + 1)],
                                       eb[BS:128, 257:258])
                    if 1 <= k <= NP - 1:
                        nc.tensor.matmul(
                            phop[1][:], sh[:],
                            eb[0:BS, 258:259], start=True, stop=True)
                        et = ebufs[(k + LAG) % NB]
                        nc.scalar.copy(et[BS:128, 0:1],
                                       phop[1][BS:128, :])
                        nc.scalar.copy(et[BS:128, 1:2],
                                       phop[1][BS:128, :])
                if k <= NSTREAM - 2:
                    if k == 0:
                        d0 = col0sk[:, 1:258]
                    else:
                        nc.vector.scalar_tensor_tensor(
                            u2[:, 0:CH], obp[:, 1:1 + CH], msk[:, k:k + 1],
                            eb[:, 1:1 + CH], op0=ALU.mult, op1=ALU.add)
                        d0 = u2[:, 0:CH]
                    nc.vector.tensor_tensor_scan(
                        ob[:, 2:2 + CH], d0, rat[:, k * CH:(k + 1) * CH],
                        ob[:, 0:1], op0=ALU.add, op1=ALU.mult)
                    if k <= NP - 2:
                        nc.tensor.matmul(
                            phop[0][:], sh[:],
                            ob[0:BS, 258:259], start=True, stop=True)
                        ot = obufs[(k + LAG) % NB]
                        nc.scalar.copy(ot[BS:128, 0:1],
                                       phop[0][BS:128, :])
                        nc.scalar.copy(ot[BS:128, 1:2],
                                       phop[0][BS:128, :])

            nc.gpsimd.dma_start(out_d[:], res[:])

    return nc


def _build_program():
    """Build the per-core Bass program (same program for all 8 cores)."""
    import concourse.bass as bass
    import concourse.mybir as mybir
    import concourse.tile as tile

    f32 = mybir.dt.float32
    b16 = mybir.dt.bfloat16
    ALU = mybir.AluOpType
    ACTF = mybir.ActivationFunctionType

    nc = bass.Bass()
    # DRAM inputs (per core)
    rat_d = nc.dram_tensor("rat", [BS, L, TP], b16, kind="ExternalInput")
    qb_d = nc.dram_tensor("qb", [BS, TP], f32, kind="ExternalInput")
    m_d = nc.dram_tensor("msk", [BS, L], f32, kind="ExternalInput")
    yt_d = nc.dram_tensor("yt", [C, BS * TP], b16, kind="ExternalInput")
    dr_d = nc.dram_tensor("dr", [BS, TP], b16, kind="ExternalInput")
    out_d = nc.dram_tensor("out", [BS, NP + 2], f32, kind="ExternalOutput")

    TCH = 4            # number of 128-wide t chunks for denom (covers t<512)

    with tile.TileContext(nc) as tc:
        with (
            tc.tile_pool(name="pool", bufs=1) as pool,
            tc.tile_pool(name="psum", bufs=1, space="PSUM") as psum,
        ):
            rat = pool.tile([BS, L * TP], b16)
            qb = pool.tile([BS, TP], f32)
            msk = pool.tile([BS, L], f32)
            yt = pool.tile([C, BS * TP], b16)
            ones96 = pool.tile([C, 1], b16)
            dr = pool.tile([BS, TP], b16)          # envelope decay row
            zrow = pool.tile([BS, TP], b16)        # zeros (col-0 scan data0)
            col0 = pool.tile([BS, 2 + TP], f32)    # all-blank column
            u = pool.tile([BS, TP], f32)
            obufs = [pool.tile([BS, 2 + TP], f32, name=f"ob{i}", tag=f"ob{i}")
                     for i in range(2)]
            ebufs = [pool.tile([BS, 2 + TP], f32, name=f"eb{i}", tag=f"eb{i}")
                     for i in range(2)]
            lnqb = pool.tile([BS, TP], f32)
            ld = [pool.tile([128, BS], f32, name=f"ld{i}", tag=f"ld{i}")
                  for i in range(TCH)]
            res = pool.tile([BS, NP + 2], f32)
            pden = [psum.tile([128, BS], f32, name=f"pd{i}", tag=f"pd{i}")
                    for i in range(TCH)]
            psums = psum.tile([BS, 1], f32)

            # --- loads ---
            nc.gpsimd.dma_start(msk[:], m_d[:])
            nc.gpsimd.dma_start(qb[:], qb_d[:])
            nc.gpsimd.dma_start(dr[:], dr_d[:])
            nc.gpsimd.dma_start(yt[:], yt_d[:])
            NRC = 10  # rat DMA chunks (10 label-rows each)
            for k in range(NRC):
                lo, hi = k * (L // NRC), (k + 1) * (L // NRC)
                nc.gpsimd.dma_start(
                    rat[:, lo * TP:hi * TP],
                    rat_d[:, lo:hi, :].rearrange("b l t -> b (l t)"))

            # --- init ---
            nc.vector.memset(zrow[:], 0.0)
            nc.vector.memset(res[:], 1.0)
            nc.vector.memset(ones96[:], 1.0)
            nc.vector.memset(col0[:], 1.0)  # pads = 1 (alpha-hat[-1,0] = 1)
            for ob in obufs:
                nc.vector.memset(ob[:], 0.0)
            for eb in ebufs:
                nc.vector.memset(eb[:], 0.0)

            # --- sum_t log qb  (ACT Ln with accum) ---
            nc.scalar.activation(lnqb[:], qb[:], ACTF.Ln,
                                 accum_out=res[:, NP:NP + 1])

            # --- denominators on PE/ACT: denom[t] = sum_c y + 96*eps ---
            eps96 = pool.tile([128, 1], f32)
            nc.vector.memset(eps96[:], float(C) * EPS)
            for k in range(TCH):
                for b in range(BS):
                    base = b * TP + 128 * k
                    nc.tensor.matmul(
                        pden[k][:, b:b + 1],
                        yt[:, base:base + 128],
                        ones96[:],
                        start=True, stop=True)
                nc.scalar.activation(ld[k][:], pden[k][:], ACTF.Ln,
                                     bias=eps96[:])
            # sum_t via matmul with a ones vector [128,1]
            ones128 = pool.tile([128, 1], f32)
            nc.vector.memset(ones128[:], 1.0)
            for k in range(TCH):
                nc.tensor.matmul(psums[:], ld[k][:], ones128[:],
                                 start=(k == 0), stop=(k == TCH - 1))
            nc.scalar.copy(res[:, NP + 1:NP + 2], psums[:])

            # --- the DP: 101 column pairs (envelope-scaled ratio space) ---
            # col 0 (all-blank): state = (0 + state) * d[t], init 1
            nc.vector.tensor_tensor_scan(
                col0[:, 2:2 + TP], zrow[:, 0:TP], dr[:, 0:TP], 1.0,
                op0=ALU.add, op1=ALU.mult)
            # pair 0: O_0 = scan(col0[t-1], r_0)
            nc.vector.tensor_tensor_scan(
                obufs[0][:, 2:2 + TP], col0[:, 1:1 + TP], rat[:, 0:TP], 0.0,
                op0=ALU.add, op1=ALU.mult)
            for i in range(1, NP):
                op, oc = obufs[(i - 1) % 2], obufs[i % 2]
                eb = ebufs[i % 2]
                # E_i: state = (O_{i-1}[j-1] + state) * d[j]
                nc.vector.tensor_tensor_scan(
                    eb[:, 2:2 + TP], op[:, 1:1 + TP], dr[:, 0:TP], 0.0,
                    op0=ALU.add, op1=ALU.mult)
                # result readout: E_i[T] is at eb slot 1+TP; copy out
                nc.scalar.copy(res[:, i:i + 1], eb[:, 1 + TP:2 + TP])
                if i <= L - 1:
                    # u[j] = E_i[j-1] + m_i * O_{i-1}[j-1]
                    nc.vector.scalar_tensor_tensor(
                        u[:, 0:TP], op[:, 1:1 + TP], msk[:, i:i + 1],
                        eb[:, 1:1 + TP],
                        op0=ALU.mult, op1=ALU.add)
                    # O_i = scan: state = (u[j] + state) * r_i[j]
                    nc.vector.tensor_tensor_scan(
                        oc[:, 2:2 + TP], u[:, 0:TP],
                        rat[:, i * TP:(i + 1) * TP], 0.0,
                        op0=ALU.add, op1=ALU.mult)

            nc.sync.dma_start(out_d[:], res[:])

    return nc


def _get_built():
    if "nc" not in _BUILT:
        _install_axon_profile_hook()
        _install_birfix()
        _BUILT["nc"] = _build_program_skew() if SKEW else _build_program()
    return _BUILT["nc"]


def _combine(outs, ll, phi_end):
    """outs: concatenated per-core 'out' arrays -> loss."""
    if SKEW:
        nc_ = outs.shape[0] if outs.ndim == 3 else outs.shape[0] // 128
        outs = outs.reshape(nc_, 128, NP + 2)
        evals = outs[:, BS:, :NP].reshape(nc_ * BS, NP)
        sums = outs[:, :BS, NP:].reshape(nc_ * BS, 2)
    else:
        outs = outs.reshape(-1, NP + 2)
        evals = outs[:, :NP]
        sums = outs[:, NP:]
    e = np.take_along_axis(evals, ll[:, None], axis=1)[:, 0]
    e = np.maximum(e, 1e-38)
    return -(np.log(e) + phi_end + sums[:, 0] - sums[:, 1]).astype(np.float32)


def _host_prep(y_true, y_pred, input_length, label_length):
    """Per-core input bundles. Pure layout/indexing prep + the blank-ratio
    division (numerics-enabling reformulation)."""
    y_true = np.asarray(y_true)
    y_pred = np.asarray(y_pred, dtype=np.float32)
    il = np.asarray(input_length).astype(np.int64)
    ll = np.asarray(label_length).astype(np.int64)

    qb_full = y_pred[:, :, BLANK] + EPS                      # [B, T]
    labv = np.take_along_axis(
        y_pred, np.clip(y_true, 0, C - 1)[:, None, :], axis=2) + EPS  # [B,T,L]
    rat = labv / qb_full[:, :, None]                         # [B, T, L]
    tmask = (np.arange(T)[None, :] < il[:, None])            # [B, T]
    vmask = (np.arange(L)[None, :] < ll[:, None])            # [B, L]
    rat *= tmask[:, :, None]
    rat *= vmask[:, None, :]
    m = np.zeros((B, L), np.float32)
    m[:, 1:] = (y_true[:, 1:] != y_true[:, :-1]).astype(np.float32)

    # --- envelope prescale: phi[b, t] = (max-plus DP max over states) - MARGIN
    # keeps the linear-space scaled DP inside fp32 range for any data.
    NEG = np.float32(-1e30)
    MARGIN = 30.0
    lrat = np.where(rat > 0, np.log(np.maximum(rat, 1e-38)), NEG)  # [B,T,L]
    M = np.full((B, L), NEG, np.float32)     # odd (label-col) Viterbi values
    Me = np.full((B, L + 1), NEG, np.float32)  # even (blank-col) values
    Me[:, 0] = 0.0
    phi = np.empty((B, T), np.float64)
    mneg = np.where(m > 0, 0.0, NEG).astype(np.float32)  # additive skip mask
    skip = np.full((B, L), NEG, np.float32)
    for t in range(T):
        lr = lrat[:, t, :]
        # odd update: max(O_j, E_j, m_j + O_{j-1}) + lr_j
        cand = np.maximum(M, Me[:, :L])
        skip[:, 1:] = M[:, :-1] + mneg[:, 1:]
        Mn = np.maximum(cand, skip) + lr
        # even update: max(E_j, O_{j-1})  (blank ratio == 1 -> +0)
        Men = Me.copy()
        Men[:, 1:] = np.maximum(Me[:, 1:], M)
        M, Me = Mn, Men
        phi[:, t] = np.maximum(M.max(1), Me.max(1))
    # The true log-sum exceeds the max-path by a path-counting "entropy gap";
    # it is almost deterministic given (label_length, t): fitted offline as
    # g = c0 + c1*logC(te, k) + c2*sqrt(te) + c3*te with te = min(t+1, il),
    # k = ll*te/il (residual spread ~ +-25 nats across samples).
    from scipy.special import gammaln
    tf = np.arange(1, T + 1)[None, :].astype(np.float64)
    te = np.minimum(tf, il[:, None].astype(np.float64))
    kk = ll[:, None].astype(np.float64) * te / np.maximum(il[:, None], 1)
    logC = gammaln(te + 1) - gammaln(kk + 1) - gammaln(te - kk + 1)
    phi += (-28.61 + 0.9188 * logC + 8.811 * np.sqrt(te) - 0.3872 * te)
    phi -= MARGIN
    # decay row d[t] = exp(phi[t-1] - phi[t]) (phi[-1] = 0); pad frame d = 1
    dphi = np.empty((B, T), np.float64)
    dphi[:, 0] = -phi[:, 0]
    dphi[:, 1:] = phi[:, :-1] - phi[:, 1:]
    edphi = np.exp(dphi).astype(np.float32)
    drow = np.ones((B, TP), dtype=bf16)
    drow[:, :T] = edphi
    phi_end = phi[:, T - 1]
    # scale the odd ratios by the same per-t factor
    rat = rat * edphi[:, :, None]

    # [B, L, T] + zero pad frame -> [B, L, TP]
    ratp = np.zeros((B, L, TP), dtype=bf16)
    ratp[:, :, :T] = rat.transpose(0, 2, 1)
    qbp = np.ones((B, TP), np.float32)
    qbp[:, :T] = np.where(tmask, qb_full, 1.0)
    # y values for denom, c-major, neutral frames beyond il and at pad
    ytp = np.empty((B, C, TP), dtype=bf16)
    ytp[:, :, :T] = y_pred.transpose(0, 2, 1)
    neutral = np.float32((1.0 - C * EPS) / C)
    dead = ~tmask  # [B, T]
    if dead.any():
        bb, tt = np.nonzero(dead)
        ytp[bb, :, tt] = neutral
    ytp[:, :, T] = neutral

    bundles = []
    if SKEW:
        # skewed layouts: rows 0..63 chunk1 of stream k, rows 64..127 chunk2
        # of stream k-LAG
        sh = np.zeros((BS, 128), np.float32)
        sh[np.arange(BS), np.arange(BS) + BS] = 1.0
        for c in range(NCORE):
            s = slice(c * BS, (c + 1) * BS)
            rp = ratp[s]          # [BS, L, TP] scaled bf16
            dw = drow[s]          # [BS, TP]
            mm = m[s]
            r2 = np.zeros((128, NSTREAM, CH), dtype=bf16)
            r2[:BS, :L, :] = rp[:, :, 0:CH]
            r2[BS:, LAG:LAG + L, 0:TP - CH] = rp[:, :, CH:TP]
            d2 = np.empty((128, CH), dtype=bf16)
            d2[:BS] = dw[:, 0:CH]
            d2[BS:, 0:TP - CH] = dw[:, CH:TP]
            d2[BS:, TP - CH:] = 1.0
            m2 = np.zeros((128, NSTREAM), np.float32)
            m2[:BS, :L] = mm
            m2[BS:, LAG:LAG + L] = mm
            bundles.append({
                "rat2": r2,
                "dr2": d2,
                "drf": np.ascontiguousarray(dw),
                "qb": np.ascontiguousarray(qbp[s]),
                "msk2": m2,
                "sh": sh,
                "yt": np.ascontiguousarray(
                    ytp[s].transpose(1, 0, 2).reshape(C, BS * TP)),
            })
    else:
        for c in range(NCORE):
            s = slice(c * BS, (c + 1) * BS)
            bundles.append({
                "rat": np.ascontiguousarray(ratp[s]),
                "qb": np.ascontiguousarray(qbp[s]),
                "msk": np.ascontiguousarray(m[s]),
                "dr": np.ascontiguousarray(drow[s]),
                "yt": np.ascontiguousarray(
                    ytp[s].transpose(1, 0, 2).reshape(C, BS * TP)),
            })
    return bundles, ll, phi_end


def kernel(y_true, y_pred, input_length, label_length):
    from concourse.bass_utils import run_bass_kernel_spmd

    nc = _get_built()
    bundles, ll, phi_end = _host_prep(y_true, y_pred, input_length, label_length)
    r = run_bass_kernel_spmd(nc, bundles, core_ids=list(range(NCORE)))
    outs = np.concatenate([r.results[c]["out"] for c in range(NCORE)], 0)
    return _combine(outs, ll, phi_end)



# revision 5
# speedup vs baseline: 1.0624x; 1.0624x over previous
"""CTC loss kernel for Trainium2 (8 NeuronCores, data-parallel over batch).

Algorithm (column-scan CTC):
  reference loss = -logaddexp(a[il-1, 2ll], a[il-1, 2ll-1]) where a = CTC
  forward DP in log space over logp = log_softmax(log(y_pred+eps)).

  Identities used:
   * log_softmax(log(q)) = log(q) - log(sum_c q), q = y_pred + eps
   * Run the DP in LINEAR space on blank-RATIOS r[t,s] = q[t,lab_s]/q[t,blank].
     Then alpha_hat[t,s] = alpha[t,s] / prod_{tau<=t} q[tau,blank]; the all-blank
     path is exactly 1, and each state accumulates at most L=100 label-ratio
     factors => fp32 range is safe with NO renormalization (an extra per-t
     envelope decay keeps it safe for bf16/any data too).
   * s-major sweep: column s of the DP over all t is a first-order affine
     recurrence  state = (u[t] + state) * r[t]  -> native DVE
     tensor_tensor_scan. Even (blank) columns have r == 1 => plain cumsum.
   * One appended all-blank pad frame makes the final blank state at t=T
     equal alpha[il-1, 2ll] + alpha[il-1, 2ll-1] (both readout terms merged),
     for ANY per-sample il, after masking ratios for t >= il to zero.
  loss = -( log(E_ll[T]) + phi_end + sum_t log qb[t] - sum_t log denom[t] ),
  the two ln-sums are evaluated on host (they are independent of the DP).

Device per core (64 samples; 128 partitions = 2-way time-chunk pipeline skew):
  stream steps k=0..104: rows 0..63 process time-chunk1 of pair k, rows
  64..127 chunk2 of pair k-LAG.  E_i = scan(O_{i-1}, d); u = stt(E_i, m_i,
  O_{i-1}); O_i = scan(u, r_i).  All DP tensors bf16 (2x DVE rate).
  Chunk-boundary states cross partitions via tiny PE shift-matmuls (LAG
  slack) writing both the scan-init and boundary-data slots in one copy.
Host: layout prep (transpose/cast/gather+ratio), envelope, ln-sums, final log.
"""
import sys
import types
import json
import numpy as np
import ml_dtypes

EPS = 1e-7
B, T, C = 512, 512, 96
L = 100
NCORE = 8
BS = B // NCORE          # 64 samples per core
TP = T + 1               # +1 all-blank pad frame
NP = L + 1               # column pairs 0..100 (E_100 readout, O_100 unused)
BLANK = C - 1

bf16 = ml_dtypes.bfloat16

CH = 257             # chunk width (chunk1: t 0..256; chunk2: t 257..512 + pad)
LAG = 4              # stream lag between chunk1 and chunk2 of a pair
NSTREAM = NP + LAG   # 105 stream steps
NB = 8               # eb/ob buffer rotation depth
NRT = 15             # rat DMA split: separate tiles (fine-grained deps)
RSTEP = (NSTREAM + NRT - 1) // NRT  # 7 stream steps per rat tile

_BUILT = {}


def _install_axon_profile_hook():
    """Make run_bass_kernel_spmd(trace=True) usable under axon (optional)."""
    try:
        if "antenv.axon_hooks" in sys.modules:
            return
        import antenv  # noqa: F401
        from trn_agent_boot.trn_boot import _ntff_profile_via_ctypes
        hook = _ntff_profile_via_ctypes('/opt/axon/libaxon_pjrt.so')
        mod = types.ModuleType("antenv.axon_hooks")
        mod.get_axon_ntff_profile_hook = lambda: hook
        mod.set_axon_ntff_profile_hook = lambda h: None
        sys.modules["antenv.axon_hooks"] = mod
    except Exception:
        pass


def _install_birfix():
    """Cap sync waits per instruction for the nix walrus_driver: insert NoOps
    carrying excess waits immediately before the instruction (same engine)."""
    import concourse.bass_utils as bu
    import concourse.bass2jax as b2j
    if getattr(bu, "_ctc_birfix", False):
        return
    orig = bu.compile_bir_kernel

    def _legalize(bir_json: bytes, limit: int = 1) -> bytes:
        bir = json.loads(bir_json)
        n = 0
        changed = False
        for fn in bir.get("functions", []):
            for blk in fn.get("blocks", []):
                out = []
                for ins in blk.get("instructions", []):
                    si = ins.get("sync_info")
                    waits = (si or {}).get("on_wait") or []
                    if len(waits) > limit:
                        extra, keep = waits[:-limit], waits[-limit:]
                        for k in range(0, len(extra), limit):
                            n += 1
                            out.append({
                                "engine": ins["engine"], "ins": [],
                                "name": f"wsplit-nop-{n}", "opcode": "NoOp",
                                "outs": [],
                                "sync_info": {"on_update": [],
                                              "on_wait": extra[k:k + limit]},
                            })
                        si["on_wait"] = keep
                        changed = True
                    out.append(ins)
                blk["instructions"] = out
        return json.dumps(bir).encode() if changed else bir_json

    def patched(bir_json, tmpdir, neff_name="file.neff"):
        return orig(_legalize(bir_json), tmpdir, neff_name)

    bu.compile_bir_kernel = patched
    b2j.compile_bir_kernel = patched
    bu._ctc_birfix = True


def _build_program():
    """Skewed build, all-bf16 DP: all DVE ops [128, CH]. Rows 0..63 process
    chunk1 of stream-pair k; rows 64..127 chunk2 of pair k-LAG. Chunk-boundary
    state crosses partitions via tiny PE shift-matmuls with LAG slack."""
    import concourse.bass as bass
    import concourse.mybir as mybir
    import concourse.tile as tile

    f32 = mybir.dt.float32
    b16 = mybir.dt.bfloat16
    ALU = mybir.AluOpType
    ACTF = mybir.ActivationFunctionType

    nc = bass.Bass()
    rat_d = nc.dram_tensor("rat2", [128, NSTREAM, CH], b16, kind="ExternalInput")
    dr2_d = nc.dram_tensor("dr2", [128, CH], b16, kind="ExternalInput")
    drf_d = nc.dram_tensor("drf", [BS, TP], b16, kind="ExternalInput")
    m_d = nc.dram_tensor("msk2", [128, NSTREAM], f32, kind="ExternalInput")
    sh_d = nc.dram_tensor("sh", [BS, 128], b16, kind="ExternalInput")
    out_d = nc.dram_tensor("out", [BS, NP], f32, kind="ExternalOutput")

    with tile.TileContext(nc) as tc:
        with (
            tc.tile_pool(name="pool", bufs=1) as pool,
            tc.tile_pool(name="psum", bufs=1, space="PSUM") as psum,
        ):
            rats = [pool.tile([128, RSTEP * CH], b16, name=f"rat{i}",
                              tag=f"rat{i}") for i in range(NRT)]
            dr2 = pool.tile([128, CH], b16)
            drf = pool.tile([BS, TP], b16)
            msk = pool.tile([128, NSTREAM], f32)
            sh = pool.tile([BS, 128], b16)
            zrow = pool.tile([BS, TP], b16)
            col0f = pool.tile([BS, 1 + TP], b16)
            col0sk = pool.tile([128, 2 + CH], b16)
            u2 = pool.tile([128, CH], b16)
            obufs = [pool.tile([128, 2 + CH], b16, name=f"ob{i}", tag=f"ob{i}")
                     for i in range(NB)]
            ebufs = [pool.tile([128, 2 + CH], b16, name=f"eb{i}", tag=f"eb{i}")
                     for i in range(NB)]
            res = pool.tile([128, NP], f32)
            pcol = psum.tile([128, CH], f32)
            phop = [psum.tile([128, 2], f32, name=f"ph{i}", tag=f"ph{i}")
                    for i in range(2)]

            # --- loads (rat first: the stream consumes it) ---
            for k in range(NRT):
                lo = k * RSTEP
                hi = min(NSTREAM, lo + RSTEP)
                nc.gpsimd.dma_start(
                    rats[k][:, 0:(hi - lo) * CH],
                    rat_d[:, lo:hi, :].rearrange("b l t -> b (l t)"))
            nc.gpsimd.dma_start(dr2[:], dr2_d[:])
            nc.gpsimd.dma_start(drf[:], drf_d[:])
            nc.gpsimd.dma_start(msk[:], m_d[:])
            nc.gpsimd.dma_start(sh[:], sh_d[:])

            # --- init ---
            nc.vector.memset(zrow[:], 0.0)
            nc.vector.memset(res[:], 1.0)
            nc.vector.memset(col0f[:, 0:1], 1.0)
            nc.vector.memset(col0sk[:], 0.0)
            for ob in obufs:
                nc.vector.memset(ob[:], 0.0)
            for eb in ebufs:
                nc.vector.memset(eb[:], 0.0)

            # --- col 0 (all-blank) full-length, then skewed assembly ---
            nc.vector.tensor_tensor_scan(
                col0f[:, 1:1 + TP], zrow[:, 0:TP], drf[:, 0:TP], 1.0,
                op0=ALU.add, op1=ALU.mult)
            nc.scalar.copy(col0sk[0:BS, 1:1 + 258], col0f[:, 0:258])
            nc.tensor.matmul(pcol[:, 0:257], sh[:], col0f[:, 257:514],
                             start=True, stop=True)
            nc.scalar.copy(col0sk[BS:128, 1:258], pcol[BS:128, 0:257])

            # --- skewed DP stream ---
            for k in range(NSTREAM):
                rt = rats[k // RSTEP]
                rtof = (k % RSTEP) * CH
                ob = obufs[k % NB]
                obp = obufs[(k - 1) % NB]
                eb = ebufs[k % NB]
                if k >= 1:
                    nc.vector.tensor_tensor_scan(
                        eb[:, 2:2 + CH], obp[:, 1:1 + CH], dr2[:, 0:CH],
                        eb[:, 0:1], op0=ALU.add, op1=ALU.mult)
                    if k >= LAG + 1:
                        nc.scalar.copy(res[BS:128, (k - LAG):(k - LAG + 1)],
                                       eb[BS:128, 257:258])
                    if 1 <= k <= NP - 1:
                        nc.tensor.matmul(
                            phop[1][:], sh[:],
                            eb[0:BS, 258:259].broadcast_to((BS, 2)),
                            start=True, stop=True)
                        et = ebufs[(k + LAG) % NB]
                        nc.scalar.copy(et[BS:128, 0:2], phop[1][BS:128, :])
                if k <= NSTREAM - 2:
                    if k == 0:
                        d0 = col0sk[:, 1:258]
                    else:
                        nc.vector.scalar_tensor_tensor(
                            u2[:, 0:CH], obp[:, 1:1 + CH], msk[:, k:k + 1],
                            eb[:, 1:1 + CH], op0=ALU.mult, op1=ALU.add)
                        d0 = u2[:, 0:CH]
                    nc.vector.tensor_tensor_scan(
                        ob[:, 2:2 + CH], d0, rt[:, rtof:rtof + CH],
                        ob[:, 0:1], op0=ALU.add, op1=ALU.mult)
                    if k <= NP - 2:
                        nc.tensor.matmul(
                            phop[0][:], sh[:],
                            ob[0:BS, 258:259].broadcast_to((BS, 2)),
                            start=True, stop=True)
                        ot = obufs[(k + LAG) % NB]
                        nc.scalar.copy(ot[BS:128, 0:2], phop[0][BS:128, :])

            nc.sync.dma_start(out_d[:], res[BS:128, :])

    return nc


def _get_built():
    if "nc" not in _BUILT:
        _install_axon_profile_hook()
        _install_birfix()
        _BUILT["nc"] = _build_program()
    return _BUILT["nc"]


def _combine(outs, ll, hostsum):
    """outs: concatenated per-core 'out' arrays [B, NP] -> loss."""
    outs = outs.reshape(-1, NP)
    e = np.take_along_axis(outs.astype(np.float64), ll[:, None], axis=1)[:, 0]
    e = np.maximum(e, 1e-38)
    return -(np.log(e) + hostsum).astype(np.float32)


def _host_prep(y_true, y_pred, input_length, label_length):
    """Per-core input bundles. Pure layout/indexing prep, the blank-ratio
    division (numerics-enabling reformulation), and the two ln-sums that are
    independent of the DP."""
    y_true = np.asarray(y_true)
    y_pred = np.asarray(y_pred, dtype=np.float32)
    il = np.asarray(input_length).astype(np.int64)
    ll = np.asarray(label_length).astype(np.int64)

    qb_full = y_pred[:, :, BLANK] + EPS                      # [B, T]
    labv = np.take_along_axis(
        y_pred, np.clip(y_true, 0, C - 1)[:, None, :], axis=2) + EPS  # [B,T,L]
    rat = labv / qb_full[:, :, None]                         # [B, T, L]
    tmask = (np.arange(T)[None, :] < il[:, None])            # [B, T]
    vmask = (np.arange(L)[None, :] < ll[:, None])            # [B, L]
    rat *= tmask[:, :, None]
    rat *= vmask[:, None, :]
    m = np.zeros((B, L), np.float32)
    m[:, 1:] = (y_true[:, 1:] != y_true[:, :-1]).astype(np.float32)

    # ln-sums (independent of the DP): sum_t log qb - sum_t log denom, t < il
    denom = y_pred.sum(axis=2, dtype=np.float64) + C * EPS   # [B, T]
    lnsum = (np.where(tmask, np.log(qb_full.astype(np.float64)), 0.0).sum(1)
             - np.where(tmask, np.log(denom), 0.0).sum(1))   # [B]

    # --- envelope prescale: phi[b, t] = (max-plus DP max over states) - MARGIN
    # keeps the linear-space scaled DP inside fp32 range for any data.
    NEG = np.float32(-1e30)
    MARGIN = 30.0
    lrat = np.where(rat > 0, np.log(np.maximum(rat, 1e-38)), NEG)  # [B,T,L]
    M = np.full((B, L), NEG, np.float32)     # odd (label-col) Viterbi values
    Me = np.full((B, L + 1), NEG, np.float32)  # even (blank-col) values
    Me[:, 0] = 0.0
    phi = np.empty((B, T), np.float64)
    mneg = np.where(m > 0, 0.0, NEG).astype(np.float32)  # additive skip mask
    skip = np.full((B, L), NEG, np.float32)
    for t in range(T):
        lr = lrat[:, t, :]
        # odd update: max(O_j, E_j, m_j + O_{j-1}) + lr_j
        cand = np.maximum(M, Me[:, :L])
        skip[:, 1:] = M[:, :-1] + mneg[:, 1:]
        Mn = np.maximum(cand, skip) + lr
        # even update: max(E_j, O_{j-1})  (blank ratio == 1 -> +0)
        Men = Me.copy()
        Men[:, 1:] = np.maximum(Me[:, 1:], M)
        M, Me = Mn, Men
        phi[:, t] = np.maximum(M.max(1), Me.max(1))
    # The true log-sum exceeds the max-path by a path-counting "entropy gap";
    # it is almost deterministic given (label_length, t): fitted offline as
    # g = c0 + c1*logC(te, k) + c2*sqrt(te) + c3*te with te = min(t+1, il),
    # k = ll*te/il (residual spread ~ +-25 nats across samples).
    from scipy.special import gammaln
    tf = np.arange(1, T + 1)[None, :].astype(np.float64)
    te = np.minimum(tf, il[:, None].astype(np.float64))
    kk = ll[:, None].astype(np.float64) * te / np.maximum(il[:, None], 1)
    logC = gammaln(te + 1) - gammaln(kk + 1) - gammaln(te - kk + 1)
    phi += (-28.61 + 0.9188 * logC + 8.811 * np.sqrt(te) - 0.3872 * te)
    phi -= MARGIN
    # decay row d[t] = exp(phi[t-1] - phi[t]) (phi[-1] = 0); pad frame d = 1
    dphi = np.empty((B, T), np.float64)
    dphi[:, 0] = -phi[:, 0]
    dphi[:, 1:] = phi[:, :-1] - phi[:, 1:]
    edphi = np.exp(dphi).astype(np.float32)
    drow = np.ones((B, TP), dtype=bf16)
    drow[:, :T] = edphi
    phi_end = phi[:, T - 1]
    # scale the odd ratios by the same per-t factor
    rat = rat * edphi[:, :, None]

    # [B, L, T] + zero pad frame -> [B, L, TP]
    ratp = np.zeros((B, L, TP), dtype=bf16)
    ratp[:, :, :T] = rat.transpose(0, 2, 1)

    hostsum = lnsum + phi_end

    bundles = []
    # skewed layouts: rows 0..63 chunk1 of stream k, rows 64..127 chunk2
    # of stream k-LAG
    sh = np.zeros((BS, 128), bf16)
    sh[np.arange(BS), np.arange(BS) + BS] = 1.0
    for c in range(NCORE):
        s = slice(c * BS, (c + 1) * BS)
        rp = ratp[s]          # [BS, L, TP] scaled bf16
        dw = drow[s]          # [BS, TP]
        mm = m[s]
        r2 = np.zeros((128, NSTREAM, CH), dtype=bf16)
        r2[:BS, :L, :] = rp[:, :, 0:CH]
        r2[BS:, LAG:LAG + L, 0:TP - CH] = rp[:, :, CH:TP]
        d2 = np.empty((128, CH), dtype=bf16)
        d2[:BS] = dw[:, 0:CH]
        d2[BS:, 0:TP - CH] = dw[:, CH:TP]
        d2[BS:, TP - CH:] = 1.0
        m2 = np.zeros((128, NSTREAM), np.float32)
        m2[:BS, :L] = mm
        m2[BS:, LAG:LAG + L] = mm
        bundles.append({
            "rat2": r2,
            "dr2": d2,
            "drf": np.ascontiguousarray(dw),
            "msk2": m2,
            "sh": sh,
        })
    return bundles, ll, hostsum


def kernel(y_true, y_pred, input_length, label_length):
    from concourse.bass_utils import run_bass_kernel_spmd

    nc = _get_built()
    bundles, ll, hostsum = _host_prep(y_true, y_pred, input_length, label_length)
    r = run_bass_kernel_spmd(nc, bundles, core_ids=list(range(NCORE)))
    outs = np.concatenate([r.results[c]["out"] for c in range(NCORE)], 0)
    return _combine(outs, ll, hostsum)


# revision 7
# speedup vs baseline: 2.4152x; 2.2735x over previous
"""CTC loss kernel for Trainium2 (8 NeuronCores, data-parallel over batch).

Algorithm (column-scan CTC):
  reference loss = -logaddexp(a[il-1, 2ll], a[il-1, 2ll-1]) where a = CTC
  forward DP in log space over logp = log_softmax(log(y_pred+eps)).

  Identities used:
   * log_softmax(log(q)) = log(q) - log(sum_c q), q = y_pred + eps
   * Run the DP in LINEAR space on blank-RATIOS r[t,s] = q[t,lab_s]/q[t,blank].
     Then alpha_hat[t,s] = alpha[t,s] / prod_{tau<=t} q[tau,blank]; the all-blank
     path is exactly 1, and each state accumulates at most L=100 label-ratio
     factors => fp32 range is safe with NO renormalization (an extra per-t
     envelope decay keeps it safe for bf16/any data too).
   * s-major sweep: column s of the DP over all t is a first-order affine
     recurrence  state = (u[t] + state) * r[t]  -> native DVE
     tensor_tensor_scan. Even (blank) columns have r == 1 => plain cumsum.
   * One appended all-blank pad frame makes the final blank state at t=T
     equal alpha[il-1, 2ll] + alpha[il-1, 2ll-1] (both readout terms merged),
     for ANY per-sample il, after masking ratios for t >= il to zero.
  loss = -( log(E_ll[T]) + phi_end + sum_t log qb[t] - sum_t log denom[t] ),
  the two ln-sums are evaluated on host (they are independent of the DP).

Device per core (64 samples; 128 partitions = 2-way time-chunk pipeline skew):
  stream steps k=0..104: rows 0..63 process time-chunk1 of pair k, rows
  64..127 chunk2 of pair k-LAG.  E_i = scan(O_{i-1}, d); u = stt(E_i, m_i,
  O_{i-1}); O_i = scan(u, r_i).  All DP tensors bf16 (2x DVE rate).
  Chunk-boundary states cross partitions via tiny PE shift-matmuls (LAG
  slack) writing both the scan-init and boundary-data slots in one copy.
Host: layout prep (transpose/cast/gather+ratio), envelope, ln-sums, final log.
"""
import sys
import types
import json
import numpy as np
import ml_dtypes

EPS = 1e-7
B, T, C = 512, 512, 96
L = 100
NCORE = 8
BS = B // NCORE          # 64 samples per core
TP = T + 1               # +1 all-blank pad frame
NP = L + 1               # column pairs 0..100 (E_100 readout, O_100 unused)
BLANK = C - 1

bf16 = ml_dtypes.bfloat16

CH = 257             # chunk width (chunk1: t 0..256; chunk2: t 257..512 + pad)
LAG = 2              # stream lag between chunk1 and chunk2 of a pair
NSTREAM = NP + LAG   # 105 stream steps
NB = 8               # eb/ob buffer rotation depth
NRT = 15             # rat DMA split: separate tiles (fine-grained deps)
RSTEP = (NSTREAM + NRT - 1) // NRT  # 7 stream steps per rat tile

_BUILT = {}


def _install_axon_profile_hook():
    """Make run_bass_kernel_spmd(trace=True) usable under axon (optional)."""
    try:
        if "antenv.axon_hooks" in sys.modules:
            return
        import antenv  # noqa: F401
        from trn_agent_boot.trn_boot import _ntff_profile_via_ctypes
        hook = _ntff_profile_via_ctypes('/opt/axon/libaxon_pjrt.so')
        mod = types.ModuleType("antenv.axon_hooks")
        mod.get_axon_ntff_profile_hook = lambda: hook
        mod.set_axon_ntff_profile_hook = lambda h: None
        sys.modules["antenv.axon_hooks"] = mod
    except Exception:
        pass


def _install_birfix():
    """Cap sync waits per instruction for the nix walrus_driver: insert NoOps
    carrying excess waits immediately before the instruction (same engine)."""
    import concourse.bass_utils as bu
    import concourse.bass2jax as b2j
    if getattr(bu, "_ctc_birfix", False):
        return
    orig = bu.compile_bir_kernel

    def _legalize(bir_json: bytes, limit: int = 1) -> bytes:
        bir = json.loads(bir_json)
        n = 0
        changed = False
        for fn in bir.get("functions", []):
            for blk in fn.get("blocks", []):
                out = []
                for ins in blk.get("instructions", []):
                    si = ins.get("sync_info")
                    waits = (si or {}).get("on_wait") or []
                    if len(waits) > limit:
                        extra, keep = waits[:-limit], waits[-limit:]
                        for k in range(0, len(extra), limit):
                            n += 1
                            out.append({
                                "engine": ins["engine"], "ins": [],
                                "name": f"wsplit-nop-{n}", "opcode": "NoOp",
                                "outs": [],
                                "sync_info": {"on_update": [],
                                              "on_wait": extra[k:k + limit]},
                            })
                        si["on_wait"] = keep
                        changed = True
                    out.append(ins)
                blk["instructions"] = out
        return json.dumps(bir).encode() if changed else bir_json

    def patched(bir_json, tmpdir, neff_name="file.neff"):
        return orig(_legalize(bir_json), tmpdir, neff_name)

    bu.compile_bir_kernel = patched
    b2j.compile_bir_kernel = patched
    bu._ctc_birfix = True


def _build_program():
    """Skewed build, all-bf16 DP: all DVE ops [128, CH]. Rows 0..63 process
    chunk1 of stream-pair k; rows 64..127 chunk2 of pair k-LAG. Chunk-boundary
    state crosses partitions via tiny PE shift-matmuls with LAG slack."""
    import concourse.bass as bass
    import concourse.mybir as mybir
    import concourse.tile as tile

    f32 = mybir.dt.float32
    b16 = mybir.dt.bfloat16
    ALU = mybir.AluOpType
    ACTF = mybir.ActivationFunctionType

    nc = bass.Bass()
    rat_d = nc.dram_tensor("rat2", [128, NSTREAM, CH], b16, kind="ExternalInput")
    dr2_d = nc.dram_tensor("dr2", [128, CH], b16, kind="ExternalInput")
    drf_d = nc.dram_tensor("drf", [BS, TP], b16, kind="ExternalInput")
    m_d = nc.dram_tensor("msk2", [128, NSTREAM], f32, kind="ExternalInput")
    sh_d = nc.dram_tensor("sh", [BS, 128], b16, kind="ExternalInput")
    out_d = nc.dram_tensor("out", [BS, NP], f32, kind="ExternalOutput")

    with tile.TileContext(nc) as tc:
        with (
            tc.tile_pool(name="pool", bufs=1) as pool,
            tc.tile_pool(name="psum", bufs=1, space="PSUM") as psum,
        ):
            rats = [pool.tile([128, RSTEP * CH], b16, name=f"rat{i}",
                              tag=f"rat{i}") for i in range(NRT)]
            dr2 = pool.tile([128, CH], b16)
            drf = pool.tile([BS, TP], b16)
            msk = pool.tile([128, NSTREAM], f32)
            sh = pool.tile([BS, 128], b16)
            zrow = pool.tile([BS, TP], b16)
            col0f = pool.tile([BS, 1 + TP], b16)
            col0sk = pool.tile([128, 2 + CH], b16)
            u2 = pool.tile([128, CH], b16)
            obufs = [pool.tile([128, 2 + CH], b16, name=f"ob{i}", tag=f"ob{i}")
                     for i in range(NB)]
            ebufs = [pool.tile([128, 2 + CH], b16, name=f"eb{i}", tag=f"eb{i}")
                     for i in range(NB)]
            res = pool.tile([128, NP], f32)
            pcol = psum.tile([128, CH], f32)
            phop = [psum.tile([128, 2], f32, name=f"ph{i}", tag=f"ph{i}")
                    for i in range(2)]

            # --- loads (small tensors first: the col0 path needs them now;
            # rat tiles follow and land progressively as the stream consumes)
            nc.gpsimd.dma_start(dr2[:], dr2_d[:])
            nc.gpsimd.dma_start(drf[:], drf_d[:])
            nc.gpsimd.dma_start(msk[:], m_d[:])
            nc.gpsimd.dma_start(sh[:], sh_d[:])
            for k in range(NRT):
                lo = k * RSTEP
                hi = min(NSTREAM, lo + RSTEP)
                nc.gpsimd.dma_start(
                    rats[k][:, 0:(hi - lo) * CH],
                    rat_d[:, lo:hi, :].rearrange("b l t -> b (l t)"))

            # --- init ---
            nc.vector.memset(zrow[:], 0.0)
            nc.vector.memset(res[:], 1.0)
            nc.vector.memset(col0f[:, 0:1], 1.0)
            nc.vector.memset(col0sk[:], 0.0)
            for ob in obufs:
                nc.vector.memset(ob[:], 0.0)
            for eb in ebufs:
                nc.vector.memset(eb[:], 0.0)

            # --- col 0 (all-blank) full-length, then skewed assembly ---
            nc.vector.tensor_tensor_scan(
                col0f[:, 1:1 + TP], zrow[:, 0:TP], drf[:, 0:TP], 1.0,
                op0=ALU.add, op1=ALU.mult)
            nc.scalar.copy(col0sk[0:BS, 1:1 + 258], col0f[:, 0:258])
            nc.tensor.matmul(pcol[:, 0:257], sh[:], col0f[:, 257:514],
                             start=True, stop=True)
            nc.scalar.copy(col0sk[BS:128, 1:258], pcol[BS:128, 0:257])

            # --- skewed DP stream ---
            for k in range(NSTREAM):
                rt = rats[k // RSTEP]
                rtof = (k % RSTEP) * CH
                ob = obufs[k % NB]
                obp = obufs[(k - 1) % NB]
                eb = ebufs[k % NB]
                if k >= 1:
                    nc.vector.tensor_tensor_scan(
                        eb[:, 2:2 + CH], obp[:, 1:1 + CH], dr2[:, 0:CH],
                        eb[:, 0:1], op0=ALU.add, op1=ALU.mult)
                    if k >= LAG + 1:
                        nc.scalar.copy(res[BS:128, (k - LAG):(k - LAG + 1)],
                                       eb[BS:128, 257:258])
                    if 1 <= k <= NP - 1:
                        nc.tensor.matmul(
                            phop[1][:], sh[:],
                            eb[0:BS, 258:259].broadcast_to((BS, 2)),
                            start=True, stop=True)
                        et = ebufs[(k + LAG) % NB]
                        nc.scalar.copy(et[BS:128, 0:2], phop[1][BS:128, :])
                if k <= NSTREAM - 2:
                    if k == 0:
                        d0 = col0sk[:, 1:258]
                    else:
                        nc.vector.scalar_tensor_tensor(
                            u2[:, 0:CH], obp[:, 1:1 + CH], msk[:, k:k + 1],
                            eb[:, 1:1 + CH], op0=ALU.mult, op1=ALU.add)
                        d0 = u2[:, 0:CH]
                    nc.vector.tensor_tensor_scan(
                        ob[:, 2:2 + CH], d0, rt[:, rtof:rtof + CH],
                        ob[:, 0:1], op0=ALU.add, op1=ALU.mult)
                    if k <= NP - 2:
                        nc.tensor.matmul(
                            phop[0][:], sh[:],
                            ob[0:BS, 258:259].broadcast_to((BS, 2)),
                            start=True, stop=True)
                        ot = obufs[(k + LAG) % NB]
                        nc.scalar.copy(ot[BS:128, 0:2], phop[0][BS:128, :])

            nc.sync.dma_start(out_d[:], res[BS:128, :])

    return nc


def _get_built():
    if "nc" not in _BUILT:
        _install_axon_profile_hook()
        _install_birfix()
        _BUILT["nc"] = _build_program()
    return _BUILT["nc"]


def _combine(outs, ll, hostsum):
    """outs: concatenated per-core 'out' arrays [B, NP] -> loss."""
    outs = outs.reshape(-1, NP)
    e = np.take_along_axis(outs.astype(np.float64), ll[:, None], axis=1)[:, 0]
    e = np.maximum(e, 1e-38)
    return -(np.log(e) + hostsum).astype(np.float32)


def _host_prep(y_true, y_pred, input_length, label_length):
    """Per-core input bundles. Pure layout/indexing prep, the blank-ratio
    division (numerics-enabling reformulation), and the two ln-sums that are
    independent of the DP."""
    y_true = np.asarray(y_true)
    y_pred = np.asarray(y_pred, dtype=np.float32)
    il = np.asarray(input_length).astype(np.int64)
    ll = np.asarray(label_length).astype(np.int64)

    qb_full = y_pred[:, :, BLANK] + EPS                      # [B, T]
    labv = np.take_along_axis(
        y_pred, np.clip(y_true, 0, C - 1)[:, None, :], axis=2) + EPS  # [B,T,L]
    rat = labv / qb_full[:, :, None]                         # [B, T, L]
    tmask = (np.arange(T)[None, :] < il[:, None])            # [B, T]
    vmask = (np.arange(L)[None, :] < ll[:, None])            # [B, L]
    rat *= tmask[:, :, None]
    rat *= vmask[:, None, :]
    m = np.zeros((B, L), np.float32)
    m[:, 1:] = (y_true[:, 1:] != y_true[:, :-1]).astype(np.float32)

    # ln-sums (independent of the DP): sum_t log qb - sum_t log denom, t < il
    denom = y_pred.sum(axis=2, dtype=np.float64) + C * EPS   # [B, T]
    lnsum = (np.where(tmask, np.log(qb_full.astype(np.float64)), 0.0).sum(1)
             - np.where(tmask, np.log(denom), 0.0).sum(1))   # [B]

    # --- envelope prescale: phi[b, t] = (max-plus DP max over states) - MARGIN
    # keeps the linear-space scaled DP inside fp32 range for any data.
    NEG = np.float32(-1e30)
    MARGIN = 30.0
    lrat = np.where(rat > 0, np.log(np.maximum(rat, 1e-38)), NEG)  # [B,T,L]
    M = np.full((B, L), NEG, np.float32)     # odd (label-col) Viterbi values
    Me = np.full((B, L + 1), NEG, np.float32)  # even (blank-col) values
    Me[:, 0] = 0.0
    phi = np.empty((B, T), np.float64)
    mneg = np.where(m > 0, 0.0, NEG).astype(np.float32)  # additive skip mask
    skip = np.full((B, L), NEG, np.float32)
    for t in range(T):
        lr = lrat[:, t, :]
        # odd update: max(O_j, E_j, m_j + O_{j-1}) + lr_j
        cand = np.maximum(M, Me[:, :L])
        skip[:, 1:] = M[:, :-1] + mneg[:, 1:]
        Mn = np.maximum(cand, skip) + lr
        # even update: max(E_j, O_{j-1})  (blank ratio == 1 -> +0)
        Men = Me.copy()
        Men[:, 1:] = np.maximum(Me[:, 1:], M)
        M, Me = Mn, Men
        phi[:, t] = np.maximum(M.max(1), Me.max(1))
    # The true log-sum exceeds the max-path by a path-counting "entropy gap";
    # it is almost deterministic given (label_length, t): fitted offline as
    # g = c0 + c1*logC(te, k) + c2*sqrt(te) + c3*te with te = min(t+1, il),
    # k = ll*te/il (residual spread ~ +-25 nats across samples).
    from scipy.special import gammaln
    tf = np.arange(1, T + 1)[None, :].astype(np.float64)
    te = np.minimum(tf, il[:, None].astype(np.float64))
    kk = ll[:, None].astype(np.float64) * te / np.maximum(il[:, None], 1)
    logC = gammaln(te + 1) - gammaln(kk + 1) - gammaln(te - kk + 1)
    phi += (-28.61 + 0.9188 * logC + 8.811 * np.sqrt(te) - 0.3872 * te)
    phi -= MARGIN
    # decay row d[t] = exp(phi[t-1] - phi[t]) (phi[-1] = 0); pad frame d = 1
    dphi = np.empty((B, T), np.float64)
    dphi[:, 0] = -phi[:, 0]
    dphi[:, 1:] = phi[:, :-1] - phi[:, 1:]
    edphi = np.exp(dphi).astype(np.float32)
    drow = np.ones((B, TP), dtype=bf16)
    drow[:, :T] = edphi
    phi_end = phi[:, T - 1]
    # scale the odd ratios by the same per-t factor
    rat = rat * edphi[:, :, None]

    # [B, L, T] + zero pad frame -> [B, L, TP]
    ratp = np.zeros((B, L, TP), dtype=bf16)
    ratp[:, :, :T] = rat.transpose(0, 2, 1)

    hostsum = lnsum + phi_end

    bundles = []
    # skewed layouts: rows 0..63 chunk1 of stream k, rows 64..127 chunk2
    # of stream k-LAG
    sh = np.zeros((BS, 128), bf16)
    sh[np.arange(BS), np.arange(BS) + BS] = 1.0
    for c in range(NCORE):
        s = slice(c * BS, (c + 1) * BS)
        rp = ratp[s]          # [BS, L, TP] scaled bf16
        dw = drow[s]          # [BS, TP]
        mm = m[s]
        r2 = np.zeros((128, NSTREAM, CH), dtype=bf16)
        r2[:BS, :L, :] = rp[:, :, 0:CH]
        r2[BS:, LAG:LAG + L, 0:TP - CH] = rp[:, :, CH:TP]
        d2 = np.empty((128, CH), dtype=bf16)
        d2[:BS] = dw[:, 0:CH]
        d2[BS:, 0:TP - CH] = dw[:, CH:TP]
        d2[BS:, TP - CH:] = 1.0
        m2 = np.zeros((128, NSTREAM), np.float32)
        m2[:BS, :L] = mm
        m2[BS:, LAG:LAG + L] = mm
        bundles.append({
            "rat2": r2,
            "dr2": d2,
            "drf": np.ascontiguousarray(dw),
            "msk2": m2,
            "sh": sh,
        })
    return bundles, ll, hostsum


def kernel(y_true, y_pred, input_length, label_length):
    from concourse.bass_utils import run_bass_kernel_spmd

    nc = _get_built()
    bundles, ll, hostsum = _host_prep(y_true, y_pred, input_length, label_length)
    r = run_bass_kernel_spmd(nc, bundles, core_ids=list(range(NCORE)))
    outs = np.concatenate([r.results[c]["out"] for c in range(NCORE)], 0)
    return _combine(outs, ll, hostsum)


# revision 8
# speedup vs baseline: 2.4548x; 1.0164x over previous
"""CTC loss kernel for Trainium2 (8 NeuronCores, data-parallel over batch).

v3: the whole per-pair DP update (blank-state scan + skip-add + label-state
scan) runs as ONE hand-written custom DVE instruction per pair per step
(2-way time-chunk skew across partitions), replacing the 3-instruction
scan/stt/scan chain.

Algorithm (column-scan CTC, linear blank-ratio space + envelope prescale):
  loss = -( log(E_ll[T]) + phi_end + sum_t log qb[t] - sum_t log denom[t] );
  ln-sums on host. Per pair i, per frame j:
      W = E + O_prev[j-1];  E' = W*d[j];  u = W + (m-1)*O_prev[j-1]
      O' = (O + u)*r_i[j]
  One fused DVE op computes both recurrences (states in the per-block A/B
  flops), seeded from a 2-element stream prefix, emitting the O column plus
  a trailing E-readout element.

Device per core (64 samples; 128 partitions = 2-way time-chunk pipeline skew):
  stream steps k=0..104: rows 0..63 chunk1 of pair k, rows 64..127 chunk2 of
  pair k-LAG. Per step: 1 fused DVE op; 1 PE shift-matmul moving the chunk
  boundary states to rows 64..127; 3 small scalar copies (seeds, boundary,
  E-readout).
Host: layout/gather/ratio + envelope + ln-sums + final log.
"""
import sys
import types
import json
import numpy as np
import ml_dtypes

EPS = 1e-7
B, T, C = 512, 512, 96
L = 100
NCORE = 8
BS = B // NCORE          # 64 samples per core
TP = T + 1               # +1 all-blank pad frame
NP = L + 1               # column pairs 0..100
BLANK = C - 1

bf16 = ml_dtypes.bfloat16

CH = 257                 # elements per fused op (chunk1: frames 0..256;
                         # chunk2: frames 257..512 + 1 dummy readout elem)
LAG = 4                  # stream lag between chunk1 and chunk2 of a pair
NSTREAM = NP + LAG       # 105 stream steps
NB = 8                   # buf rotation depth
NI1 = 2 * CH + 1         # src1 elems per op: [d0 | r0 d1 | ... | r256 d257]
NRT = 15                 # rat DMA split into separate tiles
RSTEP = (NSTREAM + NRT - 1) // NRT

_BUILT = {}


def _install_axon_profile_hook():
    """Make run_bass_kernel_spmd(trace=True) usable under axon (optional)."""
    try:
        if "antenv.axon_hooks" in sys.modules:
            return
        import antenv  # noqa: F401
        from trn_agent_boot.trn_boot import _ntff_profile_via_ctypes
        hook = _ntff_profile_via_ctypes('/opt/axon/libaxon_pjrt.so')
        mod = types.ModuleType("antenv.axon_hooks")
        mod.get_axon_ntff_profile_hook = lambda: hook
        mod.set_axon_ntff_profile_hook = lambda h: None
        sys.modules["antenv.axon_hooks"] = mod
    except Exception:
        pass


def _install_birfix():
    """Cap sync waits per instruction for the nix walrus_driver: insert NoOps
    carrying excess waits immediately before the instruction (same engine)."""
    import concourse.bass_utils as bu
    import concourse.bass2jax as b2j
    if getattr(bu, "_ctc_birfix", False):
        return
    orig = bu.compile_bir_kernel

    def _legalize(bir_json: bytes, limit: int = 1) -> bytes:
        bir = json.loads(bir_json)
        n = 0
        changed = False
        for fn in bir.get("functions", []):
            for blk in fn.get("blocks", []):
                out = []
                for ins in blk.get("instructions", []):
                    si = ins.get("sync_info")
                    waits = (si or {}).get("on_wait") or []
                    if len(waits) > limit:
                        extra, keep = waits[:-limit], waits[-limit:]
                        for k in range(0, len(extra), limit):
                            n += 1
                            out.append({
                                "engine": ins["engine"], "ins": [],
                                "name": f"wsplit-nop-{n}", "opcode": "NoOp",
                                "outs": [],
                                "sync_info": {"on_update": [],
                                              "on_wait": extra[k:k + limit]},
                            })
                        si["on_wait"] = keep
                        changed = True
                    out.append(ins)
                blk["instructions"] = out
        return json.dumps(bir).encode() if changed else bir_json

    def patched(bir_json, tmpdir, neff_name="file.neff"):
        return orig(_legalize(bir_json), tmpdir, neff_name)

    bu.compile_bir_kernel = patched
    b2j.compile_bir_kernel = patched
    bu._ctc_birfix = True


def _register_fused_op():
    """Hand-written DVE uOp program: fused CTC pair update.

    src0 = [O_seed, E_seed, Op(257)] (bf16); src1 = [d0|r0 d1|...|r256 d257]
    (bf16); s0 = (m-1) per partition. out = [O'(257), E_final] (bf16).
    FSM: seedO -> seedE -> seedD -> (V1 <-> V2)* -> bubble -> end.
      seedO/seedE: load B (blk5) / A (blk1) flops from the stream prefix.
      seedD, V2: prefetch the next d into blk1's swap flop.
      V1: consumes Op + r; W=Op+E(A); E'=W*swap_d (->A); T=(m-1)*Op;
          S1=T+W; X=S1+O(B at blk4, i.e. blk5's flop); O'=X*r (->B); out O'.
      end: emit A (E_final) as one extra out element.
    A/B flops are per-block: NEXT_ALU_OUT_[AB] at blk k reads blk k+1's flop.
    """
    import concourse.dve_ops as dve_ops
    if "op" in _BUILT:
        return _BUILT["op"]
    from concourse.dve_spec import Spec, Src0
    from concourse.dve_uop import (
        AluInp, AluOp, DelayInp, DveOpSpec, InpSel, OutPath, OutSel,
        Trigger, UopConfig, UopDpConfig, ENABLE,
    )
    from dataclasses import dataclass

    def blocks():
        return [UopDpConfig() for _ in range(8)]

    def bypass_chain(dp, lo, hi):
        for k in range(lo, hi + 1):
            dp[k].pass_through_alu()

    # seedO: src0 elem 0 -> B flop (written at blk5, read at blk4)
    seedO = UopConfig()
    seedO.enable_input(InpSel.SRC_0, 0)
    seedO.require_inp0 = ENABLE
    seedO.repeat_count = 1
    seedO.trigger = (Trigger.COUNT, Trigger.NONE, Trigger.NONE)
    seedO.next_uop = (1, 0, 0)
    dp = blocks()
    bypass_chain(dp, 0, 5)
    dp[5].alu_out_b_enable = ENABLE
    seedO.datapath_config = dp

    # seedE: src0 elem 1 -> A flop (written at blk1, read at blk0)
    seedE = UopConfig()
    seedE.enable_input(InpSel.SRC_0, 0)
    seedE.require_inp0 = ENABLE
    seedE.repeat_count = 1
    seedE.trigger = (Trigger.COUNT, Trigger.NONE, Trigger.NONE)
    seedE.next_uop = (2, 0, 0)
    dp = blocks()
    bypass_chain(dp, 0, 1)
    dp[1].alu_out_a_enable = ENABLE
    seedE.datapath_config = dp

    # seedD / V2: src1 elem -> blk1 swap flop (next element's d)
    def d_prefetch(next_main):
        u = UopConfig()
        u.enable_input(InpSel.SRC_1, 0)
        u.require_inp1 = ENABLE
        u.repeat_count = 1
        dpp = blocks()
        bypass_chain(dpp, 0, 1)
        dpp[1].swap_enable = ENABLE
        u.datapath_config = dpp
        u.trigger = (Trigger.COUNT, Trigger.NONE, Trigger.NONE)
        u.next_uop = (next_main, 0, 0)
        return u

    seedD = d_prefetch(3)

    # V1: the fused element step
    v1 = UopConfig()
    v1.enable_input(InpSel.SRC_0, 0)      # Op
    v1.enable_input(InpSel.SRC_1, 1)      # r -> chain0
    v1.enable_input(InpSel.CONST_0, 2)    # m-1 -> chain1
    v1.require_inp0 = ENABLE
    v1.require_inp1 = ENABLE
    v1.repeat_count = 1
    v1.trigger = (Trigger.COUNT, Trigger.NONE, Trigger.NONE)
    v1.next_uop = (4, 0, 0)
    dp = blocks()
    dp[0].enable_alu(AluOp.ADD, AluInp.PREV_ALU_OUT, AluInp.NEXT_ALU_OUT_A)
    dp[0].enable_delay_from_src(DelayInp.PREV_DELAY, 0)       # r
    dp[0].enable_delay_from_src(DelayInp.PREV_DELAY, 1)       # m-1
    dp[0].enable_delay_from_src(DelayInp.PREV_ALU_OUT, 3)     # Op
    dp[1].enable_alu(AluOp.MULTIPLY, AluInp.PREV_ALU_OUT, AluInp.CURR_SWAP_OUT)
    dp[1].alu_out_a_enable = ENABLE
    dp[1].enable_delay_from_src(DelayInp.PREV_ALU_OUT, 4)     # W
    dp[1].pass_through_delay(0, 1, 3)
    dp[2].enable_alu(AluOp.MULTIPLY, AluInp.PREV_DELAY_1, AluInp.PREV_DELAY_3)
    dp[2].pass_through_delay(0, 4)
    dp[3].enable_alu(AluOp.ADD, AluInp.PREV_ALU_OUT, AluInp.PREV_DELAY_4)
    dp[3].pass_through_delay(0)
    dp[4].enable_alu(AluOp.ADD, AluInp.PREV_ALU_OUT, AluInp.NEXT_ALU_OUT_B)
    dp[4].pass_through_delay(0)
    dp[5].enable_alu(AluOp.MULTIPLY, AluInp.PREV_ALU_OUT, AluInp.PREV_DELAY_0)
    dp[5].alu_out_b_enable = ENABLE
    bypass_chain(dp, 6, 7)
    v1.datapath_config = dp
    v1.enable_output(OutSel.ALU_OUT, OutPath.WR0_LO)

    v2 = d_prefetch(3)
    v2.trigger = (Trigger.SRC_TENSOR_DONE, Trigger.COUNT, Trigger.NONE)
    v2.next_uop = (5, 3, 0)

    # bubble before end (lets the final A write settle)
    bubE = UopConfig()
    bubE.repeat_count = 1
    bubE.trigger = (Trigger.COUNT, Trigger.NONE, Trigger.NONE)
    bubE.next_uop = (6, 0, 0)

    # end: emit E_final (A flop, read at blk0)
    end = UopConfig()
    end.repeat_count = 1
    end.trigger = (Trigger.COUNT, Trigger.NONE, Trigger.NONE)
    end.next_uop = (0, 0, 0)
    dp = blocks()
    dp[0].enable_alu(AluOp.BYPASS, AluInp.NEXT_ALU_OUT_A, AluInp.NEXT_ALU_OUT_A)
    bypass_chain(dp, 1, 7)
    end.datapath_config = dp
    end.enable_output(OutSel.ALU_OUT, OutPath.WR0_LO)

    uops = [seedO, seedE, seedD, v1, v2, bubE, end]
    for u in uops:
        u.validate("v3")
    hand = DveOpSpec(name="", uops=uops, rd1_en=True)
    name = f"CTCF_{hand.sha('v3')[:10]}"
    hand.name = name

    from concourse.dve_table_gen import free_opcode_rows
    used_rows = set(dve_ops._SUB_OPCODE_FOR_NAME.values())
    row = next(r for r in free_opcode_rows("TRN2") if r not in used_rows)
    hand.opcode = row

    @dataclass(frozen=True)
    class HandDveOp(dve_ops.DveOp):
        hand: object = None

        def compile(self, ver):
            assert ver == "v3", f"hand op only built for v3, got {ver}"
            return self.hand

    def _ref(in0, in1, c0, c1, c2):
        P = in0.shape[0]
        N = in0.shape[1] - 2
        O = in0[:, 0].astype(np.float32).copy()
        E = in0[:, 1].astype(np.float32).copy()
        m1 = np.asarray(c0, np.float32).reshape(P)
        dd = in1[:, 0::2].astype(np.float32)
        rr = in1[:, 1::2].astype(np.float32)
        out = np.zeros((P, N + 1), np.float32)
        for j in range(N):
            Op = in0[:, 2 + j].astype(np.float32)
            W = E + Op
            E = np.float32(W * dd[:, j])
            O = np.float32((O + W + m1 * Op) * rr[:, j])
            out[:, j] = O
        out[:, N] = E
        return out

    op = HandDveOp(name=name, spec=Spec(body=Src0, reference=_ref),
                   subdim=False, uops_sha={}, hand=hand)
    if name not in dve_ops._SUB_OPCODE_FOR_NAME:
        dve_ops.OPS.append(op)
        dve_ops._SUB_OPCODE_FOR_NAME[name] = row
        dve_ops.CUSTOM_DVE_SPECS[name] = op.spec
    _BUILT["op"] = op
    return op


def _build_program():
    """Per-core Bass program: 105 fused-op stream steps + hop matmuls."""
    import concourse.bass as bass
    import concourse.mybir as mybir
    import concourse.tile as tile

    op = _register_fused_op()

    f32 = mybir.dt.float32
    b16 = mybir.dt.bfloat16

    nc = bass.Bass()
    rat_d = nc.dram_tensor("rat2", [128, NSTREAM, NI1], b16, kind="ExternalInput")
    m_d = nc.dram_tensor("msk2", [128, NSTREAM], f32, kind="ExternalInput")
    sh_d = nc.dram_tensor("sh", [BS, 128], b16, kind="ExternalInput")
    out_d = nc.dram_tensor("out", [BS, NP], f32, kind="ExternalOutput")

    with tile.TileContext(nc) as tc:
        with (
            tc.tile_pool(name="pool", bufs=1) as pool,
            tc.tile_pool(name="psum", bufs=1, space="PSUM") as psum,
        ):
            rats = [pool.tile([128, RSTEP * NI1], b16, name=f"rat{i}",
                              tag=f"rat{i}") for i in range(NRT)]
            msk = pool.tile([128, NSTREAM], f32)
            sh = pool.tile([BS, 128], b16)
            zbuf = pool.tile([128, 261], b16)
            bufs = [pool.tile([128, 261], b16, name=f"buf{i}", tag=f"buf{i}")
                    for i in range(NB)]
            res = pool.tile([128, NSTREAM], f32)
            phop = [psum.tile([128, 2], f32, name=f"ph{i}", tag=f"ph{i}")
                    for i in range(2)]

            # --- loads (small first; rat tiles land progressively) ---
            nc.gpsimd.dma_start(msk[:], m_d[:])
            nc.gpsimd.dma_start(sh[:], sh_d[:])
            for k in range(NRT):
                lo = k * RSTEP
                hi = min(NSTREAM, lo + RSTEP)
                nc.gpsimd.dma_start(
                    rats[k][:, 0:(hi - lo) * NI1],
                    rat_d[:, lo:hi, :].rearrange("b l t -> b (l t)"))

            # --- init ---
            nc.vector.memset(zbuf[:], 0.0)
            nc.vector.memset(zbuf[0:BS, 1:2], 1.0)   # E_seed = 1 (pair 0)
            nc.vector.memset(res[:], 0.0)
            for bb in bufs:
                nc.vector.memset(bb[:], 0.0)

            # --- fused DP stream ---
            for k in range(NSTREAM):
                rt = rats[k // RSTEP]
                rtof = (k % RSTEP) * NI1
                src = zbuf if k == 0 else bufs[(k - 1) % NB]
                buf = bufs[k % NB]
                nc.vector._custom_dve(
                    op, out=buf[:, 3:261], in0=src[:, 0:259],
                    in1=rt[:, rtof:rtof + NI1],
                    s0=msk[:, k:k + 1], s1=0.0, imm2=0.0)
                if k >= LAG:
                    nc.scalar.copy(res[BS:128, k:k + 1], buf[BS:128, 260:261])
                if k <= NP - 1:
                    nc.tensor.matmul(phop[k % 2][:], sh[:],
                                     buf[0:BS, 259:261], start=True, stop=True)
                    nc.scalar.copy(bufs[(k + LAG - 1) % NB][BS:128, 0:2],
                                   phop[k % 2][BS:128, 0:2])
                    if k <= NP - 2:
                        nc.scalar.copy(bufs[(k + LAG) % NB][BS:128, 2:3],
                                       phop[k % 2][BS:128, 0:1])

            nc.gpsimd.dma_start(out_d[:], res[BS:128, LAG:LAG + NP])

    import concourse.mybir as mybir2
    mybir2.codegen_inst_isa_subclasses(nc)
    return nc


def _get_built():
    if "nc" not in _BUILT:
        _install_axon_profile_hook()
        _install_birfix()
        _BUILT["nc"] = _build_program()
    return _BUILT["nc"]


def _combine(outs, ll, hostsum):
    """outs: concatenated per-core 'out' arrays [B, NP] -> loss."""
    outs = outs.reshape(-1, NP)
    e = np.take_along_axis(outs.astype(np.float64), ll[:, None], axis=1)[:, 0]
    e = np.maximum(e, 1e-38)
    return -(np.log(e) + hostsum).astype(np.float32)


def _host_prep(y_true, y_pred, input_length, label_length):
    """Per-core input bundles: layout/indexing prep, blank-ratio division
    (numerics-enabling reformulation), envelope, and the DP-independent
    ln-sums."""
    y_true = np.asarray(y_true)
    y_pred = np.asarray(y_pred, dtype=np.float32)
    il = np.asarray(input_length).astype(np.int64)
    ll = np.asarray(label_length).astype(np.int64)

    qb_full = y_pred[:, :, BLANK] + EPS                      # [B, T]
    labv = np.take_along_axis(
        y_pred, np.clip(y_true, 0, C - 1)[:, None, :], axis=2) + EPS  # [B,T,L]
    rat = labv / qb_full[:, :, None]                         # [B, T, L]
    tmask = (np.arange(T)[None, :] < il[:, None])            # [B, T]
    vmask = (np.arange(L)[None, :] < ll[:, None])            # [B, L]
    rat *= tmask[:, :, None]
    rat *= vmask[:, None, :]
    m = np.zeros((B, L), np.float32)
    m[:, 1:] = (y_true[:, 1:] != y_true[:, :-1]).astype(np.float32)

    # ln-sums (independent of the DP): sum_t log qb - sum_t log denom, t < il
    denom = y_pred.sum(axis=2, dtype=np.float64) + C * EPS   # [B, T]
    lnsum = (np.where(tmask, np.log(qb_full.astype(np.float64)), 0.0).sum(1)
             - np.where(tmask, np.log(denom), 0.0).sum(1))   # [B]

    # --- envelope prescale: phi[b, t] = (max-plus DP max over states) - MARGIN
    NEG = np.float32(-1e30)
    MARGIN = 30.0
    lrat = np.where(rat > 0, np.log(np.maximum(rat, 1e-38)), NEG)  # [B,T,L]
    M = np.full((B, L), NEG, np.float32)
    Me = np.full((B, L + 1), NEG, np.float32)
    Me[:, 0] = 0.0
    phi = np.empty((B, T), np.float64)
    mneg = np.where(m > 0, 0.0, NEG).astype(np.float32)
    skip = np.full((B, L), NEG, np.float32)
    for t in range(T):
        lr = lrat[:, t, :]
        cand = np.maximum(M, Me[:, :L])
        skip[:, 1:] = M[:, :-1] + mneg[:, 1:]
        Mn = np.maximum(cand, skip) + lr
        Men = Me.copy()
        Men[:, 1:] = np.maximum(Me[:, 1:], M)
        M, Me = Mn, Men
        phi[:, t] = np.maximum(M.max(1), Me.max(1))
    # path-counting "entropy gap" fit (see baseline)
    from scipy.special import gammaln
    tf = np.arange(1, T + 1)[None, :].astype(np.float64)
    te = np.minimum(tf, il[:, None].astype(np.float64))
    kk = ll[:, None].astype(np.float64) * te / np.maximum(il[:, None], 1)
    logC = gammaln(te + 1) - gammaln(kk + 1) - gammaln(te - kk + 1)
    phi += (-28.61 + 0.9188 * logC + 8.811 * np.sqrt(te) - 0.3872 * te)
    phi -= MARGIN
    dphi = np.empty((B, T), np.float64)
    dphi[:, 0] = -phi[:, 0]
    dphi[:, 1:] = phi[:, :-1] - phi[:, 1:]
    edphi = np.exp(dphi).astype(np.float32)
    drow = np.ones((B, TP), np.float32)
    drow[:, :T] = edphi
    phi_end = phi[:, T - 1]
    rat = rat * edphi[:, :, None]

    # [B, L, T] + zero pad frame -> [B, L, TP]
    ratp = np.zeros((B, L, TP), np.float32)
    ratp[:, :, :T] = rat.transpose(0, 2, 1)

    hostsum = lnsum + phi_end

    bundles = []
    sh = np.zeros((BS, 128), bf16)
    sh[np.arange(BS), np.arange(BS) + BS] = 1.0
    for c in range(NCORE):
        s = slice(c * BS, (c + 1) * BS)
        rp = ratp[s]          # [BS, L, TP] f32 (scaled)
        dw = drow[s]          # [BS, TP]
        mm = m[s]
        r2 = np.zeros((128, NSTREAM, NI1), dtype=bf16)
        # chunk1 rows 0..63: frames 0..256; d-prefetch tail = dw[257]
        r2[:BS, :, 0] = dw[:, 0:1]
        r2[:BS, :, 2::2] = dw[:, None, 1:258]
        r2[:BS, :L, 1::2] = rp[:, :, 0:257]
        # chunk2 rows 64..127 (stream k = pair k-LAG): frames 257..512 +
        # dummy readout elem (d=1, r=0) + d-prefetch tail
        d2 = np.empty((BS, CH + 1), np.float32)   # d for elems 0..256 + tail
        d2[:, 0:256] = dw[:, 257:513]
        d2[:, 256] = 1.0     # dummy elem's d (readout propagation)
        d2[:, 257] = 1.0     # prefetch tail
        r2[BS:, :, 0] = d2[:, 0:1]
        r2[BS:, :, 2::2] = d2[:, None, 1:258]
        r2[BS:, LAG:LAG + L, 1::2][:, :, 0:256] = rp[:, :, 257:513]
        # elem 256 (dummy) r stays 0
        m2 = np.full((128, NSTREAM), -1.0, np.float32)
        m2[:BS, :L] = mm - 1.0
        m2[BS:, LAG:LAG + L] = mm - 1.0
        bundles.append({
            "rat2": r2,
            "msk2": m2,
            "sh": sh,
        })
    return bundles, ll, hostsum


def kernel(y_true, y_pred, input_length, label_length):
    from concourse.bass_utils import run_bass_kernel_spmd

    nc = _get_built()
    bundles, ll, hostsum = _host_prep(y_true, y_pred, input_length, label_length)
    r = run_bass_kernel_spmd(nc, bundles, core_ids=list(range(NCORE)))
    outs = np.concatenate([r.results[c]["out"] for c in range(NCORE)], 0)
    return _combine(outs, ll, hostsum)
